# revision 1
# baseline (speedup 1.0000x reference)
"""PointConv (gnn_message_passing) Bass kernel for 8 TRN2 NeuronCores.

Math (per reference):
  pos_local = pos_in[in_index] - pos_in[out_index]            [E, 3]
  deg = clamp(segment_sum(ones, out_index), 1)
  xj = x_in[in_index, 0] / deg[out_index]                     [E]
  M = celu(celu(pos_local @ W1) @ W2)                         [E, 64]
  P = segment_sum(xj[:, None] * M, out_index)                 [N, 64]
  out = P @ W3 + b3                                           [N, 64]

Sharding: out_index == repeat(arange(N), 32) (sorted, 32 edges/node), so
core d owns nodes [d*6250, (d+1)*6250) == edges [d*200000, (d+1)*200000).
Host does the index gather (halo gather per sharding hint) + layout
interleave; device does the MLP, celu, xj-scale and segment reduce.

Device layout (per core, E_loc=200000 padded to 200704 = 49*4096):
  - edges processed in 49 super-chunks of 4096 = 8 sub-chunks of 512
  - mm1: block-diag(8x W1) [24,128] x posT8[:, 512 cols] -> PSUM [128,512]
    (partition block 16i = features of sub-chunk i)
  - celu1 = max(h, min(exp(h),1)-1) via ACT Exp + DVE ts/tt
  - mm2: block-diag(2x W2) [32,128] x c1[32j:32j+32] -> PSUM [128,512]
    (partitions 0:64 = sub-chunk 2j, 64:128 = sub-chunk 2j+1)
  - xj broadcast via ones-block-diag matmul -> PSUM [128,512]
  - celu2, * xj, then strided segment reduce [128,16,32] -> PT[:, 16 cols]
  - mm3: W3 [64,64] x PT halves + bias b3 -> outT [128, 3136]
"""

import numpy as np

N = 50000
K = 32
E = N * K
NCORES = 8
N_LOC = N // NCORES          # 6250
E_LOC = E // NCORES          # 200000
SUB = 512                    # edges per sub-chunk (PSUM bank cols)
SC = 4096                    # edges per super-chunk (8 sub-chunks)
N_SC = 49                    # super-chunks per core
E_PAD = N_SC * SC            # 200704
N_TILES = E_PAD // 1024      # 196 pair-tiles
N_PAD = E_PAD // K           # 6272
OUTC = N_TILES * 16          # 3136 packed output cols
MM3_CB = 7                   # output col blocks (3136 = 7*448)
MM3_W = OUTC // MM3_CB       # 448

_CACHE = {}


def _build():
    import concourse.mybir as mybir
    import concourse.tile as tile
    from concourse import bacc

    f32 = mybir.dt.float32
    f32r = mybir.dt.float32r
    Alu = mybir.AluOpType
    Act = mybir.ActivationFunctionType

    nc = bacc.Bacc("TRN2", target_bir_lowering=False, debug=False)

    posT8 = nc.dram_tensor("posT8", (24, N_SC * SUB), f32r, kind="ExternalInput")
    xj2 = nc.dram_tensor("xj2", (2, N_TILES * SUB), f32, kind="ExternalInput")
    w1bd = nc.dram_tensor("w1bd", (24, 128), f32r, kind="ExternalInput")
    # 4 stationary variants (cols 128j): W2 blocks at rows 32j..32j+32
    w2bd = nc.dram_tensor("w2bd", (128, 512), f32r, kind="ExternalInput")
    # 2 stationary variants (cols 64h): W3 at rows 64h..64h+64
    w3d = nc.dram_tensor("w3d", (128, 128), f32r, kind="ExternalInput")
    b3d = nc.dram_tensor("b3d", (128, 1), f32, kind="ExternalInput")
    outT = nc.dram_tensor("outT", (128, OUTC), f32, kind="ExternalOutput")

    with tile.TileContext(nc) as tc:
        with (
            tc.tile_pool(name="const", bufs=1) as cpool,
            tc.tile_pool(name="data", bufs=1) as dpool,
            tc.tile_pool(name="pa", bufs=2, space="PSUM") as pa_pool,
            tc.tile_pool(name="pb", bufs=4, space="PSUM") as pb_pool,
            tc.tile_pool(name="p3", bufs=2, space="PSUM") as p3_pool,
            tc.tile_pool(name="e1p", bufs=2) as e1p,
            tc.tile_pool(name="a1p", bufs=2) as a1p,
            tc.tile_pool(name="c1p", bufs=3) as c1p,
            tc.tile_pool(name="e2p", bufs=4) as e2p,
            tc.tile_pool(name="a2p", bufs=4) as a2p,
            tc.tile_pool(name="c2p", bufs=4) as c2p,
            tc.tile_pool(name="r2p", bufs=4) as r2p,
            tc.tile_pool(name="xmp", bufs=4) as xmp,
            tc.tile_pool(name="xjp", bufs=3) as xjp,
        ):
            w1_sb = cpool.tile([24, 128], f32r)
            nc.sync.dma_start(out=w1_sb[:], in_=w1bd[:])
            w2_sb = cpool.tile([128, 512], f32r)
            nc.sync.dma_start(out=w2_sb[:], in_=w2bd[:])
            w3_sb = cpool.tile([128, 128], f32r)
            nc.sync.dma_start(out=w3_sb[:], in_=w3d[:])
            b3_sb = cpool.tile([128, 1], f32)
            nc.sync.dma_start(out=b3_sb[:], in_=b3d[:])

            pos_sb = dpool.tile([24, N_SC * SUB], f32r)
            nc.sync.dma_start(out=pos_sb[:], in_=posT8[:])
            pt = dpool.tile([128, OUTC], f32r)
            out_sb = dpool.tile([128, OUTC], f32)

            for s in range(N_SC):
                # xj broadcast: sx[p, c] = xj2[p//64, s*2048 + c]
                sx = xjp.tile([128, SC // 2], f32)
                lo, hi = s * (SC // 2), (s + 1) * (SC // 2)
                nc.sync.dma_start(
                    out=sx[0:64, :], in_=xj2[0, lo:hi].partition_broadcast(64))
                nc.sync.dma_start(
                    out=sx[64:128, :], in_=xj2[1, lo:hi].partition_broadcast(64))
                psA = pa_pool.tile([128, SUB], f32)
                nc.tensor.matmul(
                    psA[:], w1_sb[:], pos_sb[:, s * SUB:(s + 1) * SUB],
                    start=True, stop=True,
                )
                e1 = e1p.tile([128, SUB], f32)
                nc.scalar.activation(e1[:], psA[:], Act.Exp)
                a1 = a1p.tile([128, SUB], f32)
                nc.vector.tensor_scalar(
                    out=a1[:], in0=e1[:], scalar1=1.0, scalar2=1.0,
                    op0=Alu.min, op1=Alu.subtract,
                )
                c1 = c1p.tile([128, SUB], f32r)
                nc.vector.tensor_tensor(
                    out=c1[:], in0=psA[:], in1=a1[:], op=Alu.max,
                )
                for j in range(4):
                    t = 4 * s + j
                    psB = pb_pool.tile([128, SUB], f32)
                    nc.tensor.matmul(
                        psB[:], w2_sb[:, 128 * j:128 * (j + 1)], c1[:],
                        start=True, stop=True,
                    )
                    e2 = e2p.tile([128, SUB], f32)
                    nc.scalar.activation(e2[:], psB[:], Act.Exp)
                    if t % 3 == 2:
                        # D-form on DVE (PSUM-operand max is cheap there)
                        a2 = a2p.tile([128, SUB], f32)
                        nc.vector.tensor_scalar(
                            out=a2[:], in0=e2[:], scalar1=1.0, scalar2=1.0,
                            op0=Alu.min, op1=Alu.subtract,
                        )
                        c2 = c2p.tile([128, SUB], f32)
                        nc.vector.tensor_tensor(
                            out=c2[:], in0=psB[:], in1=a2[:], op=Alu.max,
                        )
                    else:
                        # celu(x) = relu(x) - relu(1 - e^x): all-ACT
                        # pointwise, only the subtract goes to Pool
                        r2 = r2p.tile([128, SUB], f32)
                        nc.scalar.activation(r2[:], psB[:], Act.Relu)
                        a2 = a2p.tile([128, SUB], f32)
                        nc.scalar.activation(
                            a2[:], e2[:], Act.Relu, scale=-1.0, bias=1.0)
                        c2 = c2p.tile([128, SUB], f32)
                        nc.gpsimd.tensor_tensor(
                            out=c2[:], in0=r2[:], in1=a2[:],
                            op=Alu.subtract,
                        )
                    xm = xmp.tile([128, SUB], f32)
                    meng = nc.gpsimd if t % 2 == 0 else nc.vector
                    meng.tensor_tensor(
                        out=xm[:], in0=c2[:],
                        in1=sx[:, j * SUB:(j + 1) * SUB], op=Alu.mult,
                    )
                    with nc.allow_low_precision(reason="f32r rounding only drops low mantissa bits"):
                        nc.vector.tensor_reduce(
                            out=pt[:, t * 16:(t + 1) * 16],
                            in_=xm[:].rearrange("p (n k) -> p n k", k=K),
                            axis=mybir.AxisListType.X, op=Alu.add,
                        )

            for h in range(2):
                for cb in range(MM3_CB):
                    ps3 = p3_pool.tile([64, MM3_W], f32)
                    nc.tensor.matmul(
                        ps3[:], w3_sb[:, 64 * h:64 * h + 64],
                        pt[:, cb * MM3_W:(cb + 1) * MM3_W],
                        start=True, stop=True,
                    )
                    nc.scalar.activation(
                        out=out_sb[64 * h:64 * h + 64,
                                   cb * MM3_W:(cb + 1) * MM3_W],
                        in_=ps3[:], func=Act.Identity,
                        bias=b3_sb[64 * h:64 * h + 64, :],
                    )
            nc.sync.dma_start(out=outT[:], in_=out_sb[:])

    nc.compile()
    return nc


def _reference_numpy(x_in, pos_in, W1, W2, W3, b3, in_index, out_index):
    def celu(x):
        return np.maximum(x, 0.0) + np.minimum(np.expm1(np.minimum(x, 0.0)), 0.0)

    pos_local = np.nan_to_num(pos_in[in_index] - pos_in[out_index])
    deg = np.bincount(out_index, minlength=N).astype(np.float32)
    deg = np.maximum(deg, 1.0)
    xj = x_in[in_index, 0] * (1.0 / deg)[out_index]
    M = celu(celu(pos_local @ W1) @ W2)
    prod = xj[:, None] * M
    P = np.zeros((N, M.shape[1]), dtype=np.float32)
    np.add.at(P, out_index, prod)
    out = P @ W3 + b3
    return np.nan_to_num(out, posinf=10000.0, neginf=-10000.0).astype(np.float32)


def _pack_weights(W1, W2, W3, b3):
    w1bd = np.zeros((24, 128), np.float32)
    for i in range(8):
        w1bd[3 * i:3 * i + 3, 16 * i:16 * i + 16] = W1
    w2bd = np.zeros((128, 512), np.float32)
    for j in range(4):
        w2bd[32 * j:32 * j + 16, 128 * j:128 * j + 64] = W2
        w2bd[32 * j + 16:32 * j + 32, 128 * j + 64:128 * j + 128] = W2
    w3d = np.zeros((128, 128), np.float32)
    w3d[0:64, 0:64] = W3
    w3d[64:128, 64:128] = W3
    b3d = np.tile(np.asarray(b3, np.float32).reshape(64, 1), (2, 1))
    return w1bd, w2bd, w3d, b3d


def build_in_maps(inputs):
    x_in = np.asarray(inputs["x_in"], dtype=np.float32)
    pos_in = np.asarray(inputs["pos_in"], dtype=np.float32)
    W1 = np.asarray(inputs["W1"], dtype=np.float32)
    W2 = np.asarray(inputs["W2"], dtype=np.float32)
    W3 = np.asarray(inputs["W3"], dtype=np.float32)
    b3 = np.asarray(inputs["b3"], dtype=np.float32)
    in_index = np.asarray(inputs["in_index"])
    out_index = np.asarray(inputs["out_index"])

    # host-side halo gather + degree normalization
    pos_local = (pos_in[in_index] - pos_in[out_index]).astype(np.float32)
    deg = np.bincount(out_index, minlength=N).astype(np.float32)
    deg = np.maximum(deg, 1.0)
    xj = (x_in[in_index, 0] * (1.0 / deg)[out_index]).astype(np.float32)

    # block-diagonal packed weights (shared by all cores)
    w1bd, w2bd, w3d, b3d = _pack_weights(W1, W2, W3, b3)

    in_maps = []
    for d in range(NCORES):
        pos_d = np.zeros((E_PAD, 3), np.float32)
        pos_d[:E_LOC] = pos_local[d * E_LOC:(d + 1) * E_LOC]
        xj_d = np.zeros((E_PAD,), np.float32)
        xj_d[:E_LOC] = xj[d * E_LOC:(d + 1) * E_LOC]
        # posT8[3i+k, s*512+j] = pos of edge s*4096 + i*512 + j, dim k
        posT8 = np.ascontiguousarray(
            pos_d.reshape(N_SC, 8, SUB, 3).transpose(1, 3, 0, 2)
            .reshape(24, N_SC * SUB))
        # xj2[h, t*512+j] = xj of edge t*1024 + h*512 + j
        xj2 = np.ascontiguousarray(
            xj_d.reshape(N_TILES, 2, SUB).transpose(1, 0, 2)
            .reshape(2, N_TILES * SUB))
        in_maps.append({
            "posT8": posT8, "xj2": xj2, "w1bd": w1bd, "w2bd": w2bd,
            "w3d": w3d, "b3d": b3d,
        })
    return in_maps


def kernel(**inputs):
    x_in = np.asarray(inputs["x_in"], dtype=np.float32)
    pos_in = np.asarray(inputs["pos_in"], dtype=np.float32)
    W1 = np.asarray(inputs["W1"], dtype=np.float32)
    W2 = np.asarray(inputs["W2"], dtype=np.float32)
    W3 = np.asarray(inputs["W3"], dtype=np.float32)
    b3 = np.asarray(inputs["b3"], dtype=np.float32)
    in_index = np.asarray(inputs["in_index"])
    out_index = np.asarray(inputs["out_index"])

    expected = np.repeat(np.arange(N, dtype=np.int64), K).astype(out_index.dtype)
    if x_in.shape != (N, 1) or not np.array_equal(out_index, expected):
        return _reference_numpy(x_in, pos_in, W1, W2, W3, b3,
                                in_index, out_index)

    in_maps = build_in_maps(inputs)

    if "nc" not in _CACHE:
        _CACHE["nc"] = _build()
    from concourse.bass_utils import run_bass_kernel_spmd
    res = run_bass_kernel_spmd(_CACHE["nc"], in_maps, list(range(NCORES)))

    out = np.empty((N, 64), np.float32)
    for d in range(NCORES):
        oT = res.results[d]["outT"]  # [128, 3136]
        # node 32t + 16h + r  <-  oT[64h + f, 16t + r]
        full = (oT.reshape(2, 64, N_TILES, 16).transpose(2, 0, 3, 1)
                .reshape(N_PAD, 64))
        out[d * N_LOC:(d + 1) * N_LOC] = full[:N_LOC]
    return np.nan_to_num(out, posinf=10000.0, neginf=-10000.0)



# revision 21
# speedup vs baseline: 1.2441x; 1.2441x over previous
"""PointConv (gnn_message_passing) Bass kernel for 8 TRN2 NeuronCores.

Math (per reference, with deg == K == 32 exactly for the standard edge list):
  pos_local = pos_in[in_index] - pos_in[out_index]            [E, 3]
  xj = x_in[in_index, 0] / 32                                 [E]
  M = celu(celu(pos_local @ W1) @ W2)                         [E, 64]
  P = segment_sum(xj[:, None] * M, out_index)                 [N, 64]
  out = P @ W3 + b3                                           [N, 64]

Device computes the shifted form (celu(x)+1 = relu(x) + min(e^x, 1)):
  c1' = celu1 + 1   (per-edge hidden, 16-wide)
  psB = c1' @ W2 = z + colsum(W2)   -> ACT bias -colsum(W2) recovers z
  c2' = celu2(z) + 1 = relu(z) + min(e^z, 1)
  pt  = segment_sum(xj * c2') = P + S_x      (S_x[n] = sum_k xj[nK+k])
  dev_out = pt @ W3 + b3 = out + S_x (x) colsum-rows(W3)
Host subtracts the rank-1 S_x (x) w3sum term (exact).

Engine split per super-chunk (4096 edges):
  ACT : exp (PSUM->SBUF fp16, bias), celu1 exp+relu, A-variant relu2
  DVE : min(e,1) [4x fp16], fused (min(e2,1)+r2) via scalar_tensor_tensor,
        B-variant relu from PSUM via tensor_scalar(sub, max), xm multiply
  Pool: segmented 32:1 sum reduce (SBUF fp16)
  PE  : mm1 (block-diag 8x W1), mm2 (single 2x W2 stationary, moving at
        partition offset 32j), mm3 at the end
  DMA : xj partition-broadcast in fp16
"""

import numpy as np

N = 50000
K = 32
E = N * K
NCORES = 8
N_LOC = N // NCORES          # 6250
E_LOC = E // NCORES          # 200000
SUB = 512
SC = 4096                    # edges per super-chunk
N_SC = 50                    # super-chunks per core (padded, even for pairs)
E_PAD = N_SC * SC            # 204800
N_TILES = E_PAD // 1024      # 200
N_PAD = E_PAD // K           # 6400
OUTC = N_SC * 64             # 3200 packed output cols
# A-variant (relu2 on ACT) for ~1/3 of SCs, else B (relu2 on DVE from PSUM)
A_VARIANT = [s % 3 == 0 for s in range(N_SC)]

_CACHE = {}


def _build():
    import concourse.mybir as mybir
    import concourse.tile as tile
    from concourse import bacc

    f32 = mybir.dt.float32
    f16 = mybir.dt.float16
    Alu = mybir.AluOpType
    Act = mybir.ActivationFunctionType
    AxisX = mybir.AxisListType.X

    nc = bacc.Bacc("TRN2", target_bir_lowering=False, debug=False)

    posT8 = nc.dram_tensor("posT8", (24, N_SC * SUB), f16, kind="ExternalInput")
    xj2 = nc.dram_tensor("xj2", (2, N_TILES * SUB), f16, kind="ExternalInput")
    w1bd = nc.dram_tensor("w1bd", (24, 128), f16, kind="ExternalInput")
    w2bd = nc.dram_tensor("w2bd", (128, 256), f16, kind="ExternalInput")
    w2csn = nc.dram_tensor("w2csn", (128, 1), f32, kind="ExternalInput")
    w2csp = nc.dram_tensor("w2csp", (128, 1), f32, kind="ExternalInput")
    w3d = nc.dram_tensor("w3d", (128, 64), f16, kind="ExternalInput")
    b3d = nc.dram_tensor("b3d", (64, 1), f32, kind="ExternalInput")
    outT = nc.dram_tensor("outT", (128, OUTC), f16, kind="ExternalOutput")

    with tile.TileContext(nc) as tc:
        with (
            tc.tile_pool(name="const", bufs=1) as cpool,
            tc.tile_pool(name="data", bufs=1) as dpool,
            tc.tile_pool(name="pa", bufs=1, space="PSUM") as pa_pool,
            tc.tile_pool(name="pb", bufs=2, space="PSUM") as pb_pool,
            tc.tile_pool(name="p3", bufs=2, space="PSUM") as p3_pool,
            tc.tile_pool(name="e1p", bufs=2) as e1p,
            tc.tile_pool(name="r1p", bufs=2) as r1p,
            tc.tile_pool(name="m1p", bufs=2) as m1p,
            tc.tile_pool(name="c1p", bufs=2) as c1p,
            tc.tile_pool(name="e2p", bufs=3) as e2p,
            tc.tile_pool(name="r2p", bufs=3) as r2p,
            tc.tile_pool(name="c2p", bufs=3) as c2p,
            tc.tile_pool(name="xmp", bufs=3) as xmp,
            tc.tile_pool(name="trp", bufs=2) as trp,
            tc.tile_pool(name="sxp", bufs=3) as sxp,
        ):
            w1_sb = cpool.tile([24, 128], f16)
            nc.sync.dma_start(out=w1_sb[:], in_=w1bd[:])
            w2_sb = cpool.tile([128, 256], f16)
            nc.sync.dma_start(out=w2_sb[:], in_=w2bd[:])
            w3_sb = cpool.tile([128, 64], f16)
            nc.sync.dma_start(out=w3_sb[:], in_=w3d[:])
            w2csn_sb = cpool.tile([128, 1], f32)
            nc.sync.dma_start(out=w2csn_sb[:], in_=w2csn[:])
            w2csp_sb = cpool.tile([128, 1], f32)
            nc.sync.dma_start(out=w2csp_sb[:], in_=w2csp[:])
            b3_sb = cpool.tile([64, 1], f32)
            nc.sync.dma_start(out=b3_sb[:], in_=b3d[:])

            pos_sb = dpool.tile([24, N_SC * SUB], f16)
            nc.sync.dma_start(out=pos_sb[:], in_=posT8[:])
            pt = dpool.tile([128, OUTC], f16)
            out_sb = dpool.tile([128, OUTC], f16)

            for p in range(N_SC // 2):
                # --- mm1 + celu1 for the SC pair (cols 0:512 = s0, 512:1024 = s1)
                psA = pa_pool.tile([128, 1024], f32)
                for q in range(2):
                    s = 2 * p + q
                    nc.tensor.matmul(
                        psA[:, 512 * q:512 * (q + 1)], w1_sb[:],
                        pos_sb[:, s * SUB:(s + 1) * SUB],
                        start=True, stop=True,
                    )
                e1 = e1p.tile([128, 1024], f16)
                nc.scalar.activation(e1[:], psA[:], Act.Exp)
                r1 = r1p.tile([128, 1024], f16)
                nc.scalar.activation(r1[:], psA[:], Act.Relu)
                m1 = m1p.tile([128, 1024], f16)
                nc.vector.tensor_scalar_min(out=m1[:], in0=e1[:], scalar1=1.0)
                c1 = c1p.tile([128, 1024], f16)
                nc.vector.tensor_add(out=c1[:], in0=r1[:], in1=m1[:])

                for q in range(2):
                    s = 2 * p + q
                    # xj broadcast tile for this SC
                    sx = sxp.tile([128, SC // 2], f16)
                    lo, hi = s * (SC // 2), (s + 1) * (SC // 2)
                    nc.sync.dma_start(
                        out=sx[0:64, :],
                        in_=xj2[0, lo:hi].partition_broadcast(64))
                    nc.sync.dma_start(
                        out=sx[64:128, :],
                        in_=xj2[1, lo:hi].partition_broadcast(64))

                    e2 = e2p.tile([128, 2048], f16)
                    if A_VARIANT[s]:
                        r2 = r2p.tile([128, 2048], f16)
                    for half in range(2):
                        psB = pb_pool.tile([128, 1024], f32)
                        for jj in range(2):
                            j = 2 * half + jj
                            g, v = j // 2, j % 2
                            nc.tensor.matmul(
                                psB[:, 512 * jj:512 * (jj + 1)],
                                w2_sb[64 * g:64 * (g + 1),
                                      128 * v:128 * (v + 1)],
                                c1[64 * g:64 * (g + 1),
                                   512 * q:512 * (q + 1)],
                                start=True, stop=True,
                            )
                        cols = slice(1024 * half, 1024 * (half + 1))
                        nc.scalar.activation(
                            e2[:, cols], psB[:], Act.Exp, bias=w2csn_sb[:])
                        if A_VARIANT[s]:
                            nc.scalar.activation(
                                r2[:, cols], psB[:], Act.Relu,
                                bias=w2csn_sb[:])
                        else:
                            if half == 0:
                                r2 = r2p.tile([128, 2048], f16, name="r2b")
                            nc.vector.tensor_scalar(
                                out=r2[:, cols], in0=psB[:],
                                scalar1=w2csp_sb[:], scalar2=0.0,
                                op0=Alu.subtract, op1=Alu.max,
                            )
                    # c2' = min(e2, 1) + r2  (fused on DVE)
                    c2 = c2p.tile([128, 2048], f16)
                    nc.vector.scalar_tensor_tensor(
                        out=c2[:], in0=e2[:], scalar=1.0, in1=r2[:],
                        op0=Alu.min, op1=Alu.add,
                    )
                    xm = xmp.tile([128, 2048], f16)
                    nc.vector.tensor_mul(out=xm[:], in0=c2[:], in1=sx[:])
                    # 32:1 segmented sum: fp16 tree; level 1 on Pool, rest DVE
                    tsum = xm
                    for width in (16, 8, 4, 2, 1):
                        seg = tsum[:].rearrange("p (g k) -> p g k", k=2 * width)
                        if width > 1:
                            nxt = trp.tile([128, 64 * width], f16)
                            dst = nxt[:].rearrange("p (g k) -> p g k", k=width)
                        else:
                            nxt = None
                            dst = (pt[:, s * 64:(s + 1) * 64]
                                   .rearrange("p (g k) -> p g k", k=width))
                        eng = nc.gpsimd if width == 16 else nc.vector
                        eng.tensor_add(
                            out=dst, in0=seg[:, :, 0:width],
                            in1=seg[:, :, width:2 * width],
                        )
                        tsum = nxt

            # --- mm3: out = pt @ W3 + b3 (both partition halves, 512-col chunks)
            n_cb = OUTC // 512 + (1 if OUTC % 512 else 0)
            for h in range(2):
                for cb in range(n_cb):
                    w = min(512, OUTC - cb * 512)
                    ps3 = p3_pool.tile([64, 512], f32)
                    nc.tensor.matmul(
                        ps3[:, 0:w], w3_sb[64 * h:64 * (h + 1), :],
                        pt[64 * h:64 * (h + 1), cb * 512:cb * 512 + w],
                        start=True, stop=True,
                    )
                    nc.scalar.activation(
                        out=out_sb[64 * h:64 * (h + 1), cb * 512:cb * 512 + w],
                        in_=ps3[:, 0:w], func=Act.Identity, bias=b3_sb[:],
                    )
            nc.sync.dma_start(out=outT[:], in_=out_sb[:])

    nc.compile()
    return nc


def _reference_numpy(x_in, pos_in, W1, W2, W3, b3, in_index, out_index):
    def celu(x):
        return np.maximum(x, 0.0) + np.minimum(np.expm1(np.minimum(x, 0.0)), 0.0)

    n = pos_in.shape[0]
    pos_local = np.nan_to_num(pos_in[in_index] - pos_in[out_index])
    deg = np.bincount(out_index, minlength=n).astype(np.float32)
    deg = np.maximum(deg, 1.0)
    xj = x_in[in_index, 0] * (1.0 / deg)[out_index]
    M = celu(celu(pos_local @ W1) @ W2)
    prod = xj[:, None] * M
    P = np.zeros((n, M.shape[1]), dtype=np.float32)
    np.add.at(P, out_index, prod)
    out = P @ W3 + b3
    return np.nan_to_num(out, posinf=10000.0, neginf=-10000.0).astype(np.float32)


def build_in_maps(inputs):
    x_in = np.asarray(inputs["x_in"], dtype=np.float32)
    pos_in = np.asarray(inputs["pos_in"], dtype=np.float32)
    W1 = np.asarray(inputs["W1"], dtype=np.float32)
    W2 = np.asarray(inputs["W2"], dtype=np.float32)
    W3 = np.asarray(inputs["W3"], dtype=np.float32)
    b3 = np.asarray(inputs["b3"], dtype=np.float32)
    in_index = np.asarray(inputs["in_index"])
    out_index = np.asarray(inputs["out_index"])

    pos_local = (pos_in[in_index] - pos_in[out_index]).astype(np.float16)
    xj = (x_in[in_index, 0] * (1.0 / K)).astype(np.float16)

    w1bd = np.zeros((24, 128), np.float16)
    for i in range(8):
        w1bd[3 * i:3 * i + 3, 16 * i:16 * i + 16] = W1
    # stationary variants: mm2 for quarter j = 2g+v uses rows 64g:64g+64,
    # cols 128v:128v+128; active contraction rows are 32v:32v+32 of the group
    w2bd = np.zeros((128, 256), np.float16)
    for g in range(2):
        for v in range(2):
            for a in range(2):
                w2bd[64 * g + 32 * v + 16 * a:64 * g + 32 * v + 16 * (a + 1),
                     128 * v + 64 * a:128 * v + 64 * (a + 1)] = W2
    w2cs = np.tile(W2.sum(axis=0).astype(np.float32).reshape(64, 1), (2, 1))
    w3d = np.tile(W3.astype(np.float16), (2, 1))
    b3d = np.asarray(b3, np.float32).reshape(64, 1)

    in_maps = []
    for d in range(NCORES):
        pos_d = np.zeros((E_PAD, 3), np.float16)
        pos_d[:E_LOC] = pos_local[d * E_LOC:(d + 1) * E_LOC]
        xj_d = np.zeros((E_PAD,), np.float16)
        xj_d[:E_LOC] = xj[d * E_LOC:(d + 1) * E_LOC]
        posT8 = np.ascontiguousarray(
            pos_d.reshape(N_SC, 8, SUB, 3).transpose(1, 3, 0, 2)
            .reshape(24, N_SC * SUB))
        xj2 = np.ascontiguousarray(
            xj_d.reshape(N_TILES, 2, SUB).transpose(1, 0, 2)
            .reshape(2, N_TILES * SUB))
        in_maps.append({
            "posT8": posT8, "xj2": xj2, "w1bd": w1bd, "w2bd": w2bd,
            "w2csn": -w2cs, "w2csp": w2cs, "w3d": w3d, "b3d": b3d,
        })
    return in_maps


def kernel(**inputs):
    x_in = np.asarray(inputs["x_in"], dtype=np.float32)
    pos_in = np.asarray(inputs["pos_in"], dtype=np.float32)
    W1 = np.asarray(inputs["W1"], dtype=np.float32)
    W2 = np.asarray(inputs["W2"], dtype=np.float32)
    W3 = np.asarray(inputs["W3"], dtype=np.float32)
    b3 = np.asarray(inputs["b3"], dtype=np.float32)
    in_index = np.asarray(inputs["in_index"])
    out_index = np.asarray(inputs["out_index"])

    expected = np.repeat(np.arange(N, dtype=np.int64), K).astype(out_index.dtype)
    if x_in.shape != (N, 1) or not np.array_equal(out_index, expected):
        return _reference_numpy(x_in, pos_in, W1, W2, W3, b3,
                                in_index, out_index)

    in_maps = build_in_maps(inputs)

    if "nc" not in _CACHE:
        _CACHE["nc"] = _build()
    from concourse.bass_utils import run_bass_kernel_spmd
    res = run_bass_kernel_spmd(_CACHE["nc"], in_maps, list(range(NCORES)))

    # host-side rank-1 correction: dev_out = out + S_x (x) w3sum
    S_x = (x_in[in_index, 0].astype(np.float64).reshape(N, K).sum(axis=1)
           / K).astype(np.float32)
    w3sum = W3.sum(axis=0).astype(np.float32)

    out = np.empty((N, 64), np.float32)
    for d in range(NCORES):
        oT = res.results[d]["outT"].astype(np.float32)  # [128, 3200]
        # col s*64 + j*16 + n_l, partition 64h+f <- node s*128 + (2j+h)*16 + n_l
        full = (oT.reshape(2, 64, N_SC, 4, 16).transpose(2, 3, 0, 4, 1)
                .reshape(N_PAD, 64))
        out[d * N_LOC:(d + 1) * N_LOC] = full[:N_LOC]
    out -= S_x[:, None] * w3sum[None, :]
    return np.nan_to_num(out, posinf=10000.0, neginf=-10000.0)


# revision 30
# speedup vs baseline: 1.6657x; 1.3388x over previous
"""PointConv (gnn_message_passing) Bass kernel for 8 TRN2 NeuronCores.

Math (per reference, with deg == K == 32 exactly for the standard edge list):
  pos_local = pos_in[in_index] - pos_in[out_index]            [E, 3]
  xj = x_in[in_index, 0] / 32                                 [E]
  M = celu(celu(pos_local @ W1) @ W2)                         [E, 64]
  P = segment_sum(xj[:, None] * M, out_index)                 [N, 64]
  out = P @ W3 + b3                                           [N, 64]

Device computes the shifted form (celu(x)+1 = relu(x) + min(e^x, 1)):
  c1' = celu1 + 1   (per-edge hidden, 16-wide)
  psB = c1' @ W2 = z + colsum(W2)   -> ACT bias -colsum(W2) recovers z
  c2' = celu2(z) + 1 = relu(z) + min(e^z, 1)
  pt  = segment_sum(xj * c2') = P + S_x      (S_x[n] = sum_k xj[nK+k])
  dev_out = pt @ W3 + b3 = out + S_x (x) colsum-rows(W3)
Host subtracts the rank-1 S_x (x) w3sum term (exact).

Engine split per super-chunk (4096 edges):
  ACT : exp (PSUM->SBUF fp16, bias), celu1 exp+relu, A-variant relu2
  DVE : min(e,1) [4x fp16], fused (min(e2,1)+r2) via scalar_tensor_tensor,
        B-variant relu from PSUM via tensor_scalar(sub, max), xm multiply
  Pool: segmented 32:1 sum reduce (SBUF fp16)
  PE  : mm1 (block-diag 8x W1), mm2 (single 2x W2 stationary, moving at
        partition offset 32j), mm3 at the end
  DMA : xj partition-broadcast in fp16
"""

import numpy as np

N = 50000
K = 32
E = N * K
NCORES = 8
N_LOC = N // NCORES          # 6250
E_LOC = E // NCORES          # 200000
SUB = 512
SC = 4096                    # edges per super-chunk
N_SC = 50                    # super-chunks per core (padded, even for pairs)
E_PAD = N_SC * SC            # 204800
N_TILES = E_PAD // 1024      # 200
N_PAD = E_PAD // K           # 6400
OUTC = N_SC * 64             # 3200 packed output cols
# A-variant (relu2 on ACT) for most SCs, else B (linear path on DVE from PSUM)
A_VARIANT = [s % 6 != 5 for s in range(N_SC)]

_CACHE = {}


def _build():
    import concourse.mybir as mybir
    import concourse.tile as tile
    from concourse import bacc

    f32 = mybir.dt.float32
    f16 = mybir.dt.float16
    Alu = mybir.AluOpType
    Act = mybir.ActivationFunctionType
    AxisX = mybir.AxisListType.X

    nc = bacc.Bacc("TRN2", target_bir_lowering=False, debug=False)

    posT8 = nc.dram_tensor("posT8", (24, N_SC * SUB), f16, kind="ExternalInput")
    xj2 = nc.dram_tensor("xj2", (2, N_TILES * SUB), f16, kind="ExternalInput")
    w1bd = nc.dram_tensor("w1bd", (24, 128), f16, kind="ExternalInput")
    w2bd = nc.dram_tensor("w2bd", (128, 256), f16, kind="ExternalInput")
    w2csn = nc.dram_tensor("w2csn", (128, 1), f32, kind="ExternalInput")
    w2csm1 = nc.dram_tensor("w2csm1", (128, 1), f32, kind="ExternalInput")
    w3d = nc.dram_tensor("w3d", (128, 64), f16, kind="ExternalInput")
    b3d = nc.dram_tensor("b3d", (64, 1), f32, kind="ExternalInput")
    outT = nc.dram_tensor("outT", (128, OUTC), f16, kind="ExternalOutput")

    with tile.TileContext(nc) as tc:
        with (
            tc.tile_pool(name="const", bufs=1) as cpool,
            tc.tile_pool(name="data", bufs=1) as dpool,
            tc.tile_pool(name="pa", bufs=1, space="PSUM") as pa_pool,
            tc.tile_pool(name="pb", bufs=2, space="PSUM") as pb_pool,
            tc.tile_pool(name="p3", bufs=2, space="PSUM") as p3_pool,
            tc.tile_pool(name="e1p", bufs=2) as e1p,
            tc.tile_pool(name="r1p", bufs=2) as r1p,
            tc.tile_pool(name="m1p", bufs=2) as m1p,
            tc.tile_pool(name="c1p", bufs=2) as c1p,
            tc.tile_pool(name="e2p", bufs=3) as e2p,
            tc.tile_pool(name="r2p", bufs=3) as r2p,
            tc.tile_pool(name="c2p", bufs=3) as c2p,
            tc.tile_pool(name="xmp", bufs=3) as xmp,
            tc.tile_pool(name="trp", bufs=2) as trp,
            tc.tile_pool(name="sxp", bufs=3) as sxp,
        ):
            w1_sb = cpool.tile([24, 128], f16)
            nc.sync.dma_start(out=w1_sb[:], in_=w1bd[:])
            w2_sb = cpool.tile([128, 256], f16)
            nc.sync.dma_start(out=w2_sb[:], in_=w2bd[:])
            w3_sb = cpool.tile([128, 64], f16)
            nc.sync.dma_start(out=w3_sb[:], in_=w3d[:])
            w2csn_sb = cpool.tile([128, 1], f32)
            nc.sync.dma_start(out=w2csn_sb[:], in_=w2csn[:])
            w2csm1_sb = cpool.tile([128, 1], f32)
            nc.sync.dma_start(out=w2csm1_sb[:], in_=w2csm1[:])
            b3_sb = cpool.tile([64, 1], f32)
            nc.sync.dma_start(out=b3_sb[:], in_=b3d[:])

            pos_sb = dpool.tile([24, N_SC * SUB], f16)
            nc.sync.dma_start(out=pos_sb[:], in_=posT8[:])
            pt = dpool.tile([128, OUTC], f16)
            out_sb = dpool.tile([128, OUTC], f16)

            for p in range(N_SC // 2):
                # --- mm1 + celu1 for the SC pair (cols 0:512 = s0, 512:1024 = s1)
                psA = pa_pool.tile([128, 1024], f32)
                for q in range(2):
                    s = 2 * p + q
                    nc.tensor.matmul(
                        psA[:, 512 * q:512 * (q + 1)], w1_sb[:],
                        pos_sb[:, s * SUB:(s + 1) * SUB],
                        start=True, stop=True,
                    )
                e1 = e1p.tile([128, 1024], f16)
                nc.scalar.activation(e1[:], psA[:], Act.Exp)
                r1 = r1p.tile([128, 1024], f16)
                nc.scalar.activation(r1[:], psA[:], Act.Relu)
                m1 = m1p.tile([128, 1024], f16)
                nc.vector.tensor_scalar_min(out=m1[:], in0=e1[:], scalar1=1.0)
                c1 = c1p.tile([128, 1024], f16)
                nc.vector.tensor_add(out=c1[:], in0=r1[:], in1=m1[:])

                for q in range(2):
                    s = 2 * p + q
                    # xj broadcast tile for this SC
                    sx = sxp.tile([128, SC // 2], f16)
                    lo, hi = s * (SC // 2), (s + 1) * (SC // 2)
                    nc.sync.dma_start(
                        out=sx[0:64, :],
                        in_=xj2[0, lo:hi].partition_broadcast(64))
                    nc.sync.dma_start(
                        out=sx[64:128, :],
                        in_=xj2[1, lo:hi].partition_broadcast(64))

                    e2 = e2p.tile([128, 2048], f16)
                    if A_VARIANT[s]:
                        r2 = r2p.tile([128, 2048], f16)
                    else:
                        r2 = r2p.tile([128, 2048], f16, name="r2b")
                    for half in range(2):
                        psB = pb_pool.tile([128, 1024], f32)
                        for jj in range(2):
                            j = 2 * half + jj
                            g, v = j // 2, j % 2
                            nc.tensor.matmul(
                                psB[:, 512 * jj:512 * (jj + 1)],
                                w2_sb[64 * g:64 * (g + 1),
                                      128 * v:128 * (v + 1)],
                                c1[64 * g:64 * (g + 1),
                                   512 * q:512 * (q + 1)],
                                start=True, stop=True,
                            )
                        cols = slice(1024 * half, 1024 * (half + 1))
                        nc.scalar.activation(
                            e2[:, cols], psB[:], Act.Exp, bias=w2csn_sb[:])
                        if A_VARIANT[s]:
                            nc.scalar.activation(
                                r2[:, cols], psB[:], Act.Relu,
                                bias=w2csn_sb[:])
                        else:
                            # r2 = max(z + 1, 1)  (linear path off PSUM)
                            nc.vector.tensor_scalar(
                                out=r2[:, cols], in0=psB[:],
                                scalar1=w2csm1_sb[:], scalar2=1.0,
                                op0=Alu.subtract, op1=Alu.max,
                            )
                    c2 = c2p.tile([128, 2048], f16)
                    if A_VARIANT[s]:
                        # c2' = min(e2, 1) + r2
                        m2 = trp.tile([128, 2048], f16, name="m2")
                        nc.vector.tensor_scalar_min(
                            out=m2[:], in0=e2[:], scalar1=1.0)
                        nc.vector.tensor_add(out=c2[:], in0=r2[:], in1=m2[:])
                    else:
                        # c2' = min(e2, max(z+1, 1))  (exact identity)
                        nc.vector.tensor_tensor(
                            out=c2[:], in0=e2[:], in1=r2[:], op=Alu.min)
                    xm = xmp.tile([128, 2048], f16)
                    nc.vector.tensor_mul(out=xm[:], in0=c2[:], in1=sx[:])
                    # 32:1 segmented sum, k-major edge order -> contiguous
                    # halves per 512-block; all levels DVE fp16 2x
                    tsum = xm
                    for width in (256, 128, 64, 32, 16):
                        seg = tsum[:].rearrange("p (g k) -> p g k", k=2 * width)
                        if width > 16:
                            nxt = trp.tile([128, 4 * width], f16,
                                           name=f"tr{width}")
                            dst = nxt[:].rearrange("p (g k) -> p g k", k=width)
                        else:
                            nxt = None
                            dst = (pt[:, s * 64:(s + 1) * 64]
                                   .rearrange("p (g k) -> p g k", k=width))
                        nc.vector.tensor_add(
                            out=dst, in0=seg[:, :, 0:width],
                            in1=seg[:, :, width:2 * width],
                        )
                        tsum = nxt

            # --- mm3: out = pt @ W3 + b3 (both partition halves, 512-col chunks)
            n_cb = OUTC // 512 + (1 if OUTC % 512 else 0)
            for h in range(2):
                for cb in range(n_cb):
                    w = min(512, OUTC - cb * 512)
                    ps3 = p3_pool.tile([64, 512], f32)
                    nc.tensor.matmul(
                        ps3[:, 0:w], w3_sb[64 * h:64 * (h + 1), :],
                        pt[64 * h:64 * (h + 1), cb * 512:cb * 512 + w],
                        start=True, stop=True,
                    )
                    nc.scalar.activation(
                        out=out_sb[64 * h:64 * (h + 1), cb * 512:cb * 512 + w],
                        in_=ps3[:, 0:w], func=Act.Identity, bias=b3_sb[:],
                    )
            nc.sync.dma_start(out=outT[:], in_=out_sb[:])

    nc.compile()
    return nc


def _reference_numpy(x_in, pos_in, W1, W2, W3, b3, in_index, out_index):
    def celu(x):
        return np.maximum(x, 0.0) + np.minimum(np.expm1(np.minimum(x, 0.0)), 0.0)

    n = pos_in.shape[0]
    pos_local = np.nan_to_num(pos_in[in_index] - pos_in[out_index])
    deg = np.bincount(out_index, minlength=n).astype(np.float32)
    deg = np.maximum(deg, 1.0)
    xj = x_in[in_index, 0] * (1.0 / deg)[out_index]
    M = celu(celu(pos_local @ W1) @ W2)
    prod = xj[:, None] * M
    P = np.zeros((n, M.shape[1]), dtype=np.float32)
    np.add.at(P, out_index, prod)
    out = P @ W3 + b3
    return np.nan_to_num(out, posinf=10000.0, neginf=-10000.0).astype(np.float32)


def build_in_maps(inputs):
    x_in = np.asarray(inputs["x_in"], dtype=np.float32)
    pos_in = np.asarray(inputs["pos_in"], dtype=np.float32)
    W1 = np.asarray(inputs["W1"], dtype=np.float32)
    W2 = np.asarray(inputs["W2"], dtype=np.float32)
    W3 = np.asarray(inputs["W3"], dtype=np.float32)
    b3 = np.asarray(inputs["b3"], dtype=np.float32)
    in_index = np.asarray(inputs["in_index"])
    out_index = np.asarray(inputs["out_index"])

    pos_local = (pos_in[in_index] - pos_in[out_index]).astype(np.float16)
    xj = (x_in[in_index, 0] * (1.0 / K)).astype(np.float16)

    w1bd = np.zeros((24, 128), np.float16)
    for i in range(8):
        w1bd[3 * i:3 * i + 3, 16 * i:16 * i + 16] = W1
    # stationary variants: mm2 for quarter j = 2g+v uses rows 64g:64g+64,
    # cols 128v:128v+128; active contraction rows are 32v:32v+32 of the group
    w2bd = np.zeros((128, 256), np.float16)
    for g in range(2):
        for v in range(2):
            for a in range(2):
                w2bd[64 * g + 32 * v + 16 * a:64 * g + 32 * v + 16 * (a + 1),
                     128 * v + 64 * a:128 * v + 64 * (a + 1)] = W2
    w2cs = np.tile(W2.sum(axis=0).astype(np.float32).reshape(64, 1), (2, 1))
    w3d = np.tile(W3.astype(np.float16), (2, 1))
    b3d = np.asarray(b3, np.float32).reshape(64, 1)

    in_maps = []
    for d in range(NCORES):
        pos_d = np.zeros((E_PAD, 3), np.float16)
        pos_d[:E_LOC] = pos_local[d * E_LOC:(d + 1) * E_LOC]
        xj_d = np.zeros((E_PAD,), np.float16)
        xj_d[:E_LOC] = xj[d * E_LOC:(d + 1) * E_LOC]
        # k-major order within each 512-edge block: col = k*16 + n_local
        pos_d = (pos_d.reshape(-1, 16, K, 3).transpose(0, 2, 1, 3)
                 .reshape(E_PAD, 3))
        xj_d = xj_d.reshape(-1, 16, K).transpose(0, 2, 1).reshape(E_PAD)
        posT8 = np.ascontiguousarray(
            pos_d.reshape(N_SC, 8, SUB, 3).transpose(1, 3, 0, 2)
            .reshape(24, N_SC * SUB))
        xj2 = np.ascontiguousarray(
            xj_d.reshape(N_TILES, 2, SUB).transpose(1, 0, 2)
            .reshape(2, N_TILES * SUB))
        in_maps.append({
            "posT8": posT8, "xj2": xj2, "w1bd": w1bd, "w2bd": w2bd,
            "w2csn": -w2cs, "w2csm1": w2cs - 1.0, "w3d": w3d, "b3d": b3d,
        })
    return in_maps


def kernel(**inputs):
    x_in = np.asarray(inputs["x_in"], dtype=np.float32)
    pos_in = np.asarray(inputs["pos_in"], dtype=np.float32)
    W1 = np.asarray(inputs["W1"], dtype=np.float32)
    W2 = np.asarray(inputs["W2"], dtype=np.float32)
    W3 = np.asarray(inputs["W3"], dtype=np.float32)
    b3 = np.asarray(inputs["b3"], dtype=np.float32)
    in_index = np.asarray(inputs["in_index"])
    out_index = np.asarray(inputs["out_index"])

    expected = np.repeat(np.arange(N, dtype=np.int64), K).astype(out_index.dtype)
    if x_in.shape != (N, 1) or not np.array_equal(out_index, expected):
        return _reference_numpy(x_in, pos_in, W1, W2, W3, b3,
                                in_index, out_index)

    in_maps = build_in_maps(inputs)

    if "nc" not in _CACHE:
        _CACHE["nc"] = _build()
    from concourse.bass_utils import run_bass_kernel_spmd
    res = run_bass_kernel_spmd(_CACHE["nc"], in_maps, list(range(NCORES)))

    # host-side rank-1 correction: dev_out = out + S_x (x) w3sum
    S_x = (x_in[in_index, 0].astype(np.float64).reshape(N, K).sum(axis=1)
           / K).astype(np.float32)
    w3sum = W3.sum(axis=0).astype(np.float32)

    out = np.empty((N, 64), np.float32)
    for d in range(NCORES):
        oT = res.results[d]["outT"].astype(np.float32)  # [128, 3200]
        # col s*64 + j*16 + n_l, partition 64h+f <- node s*128 + (2j+h)*16 + n_l
        full = (oT.reshape(2, 64, N_SC, 4, 16).transpose(2, 3, 0, 4, 1)
                .reshape(N_PAD, 64))
        out[d * N_LOC:(d + 1) * N_LOC] = full[:N_LOC]
    out -= S_x[:, None] * w3sum[None, :]
    return np.nan_to_num(out, posinf=10000.0, neginf=-10000.0)
